# revision 19
# baseline (speedup 1.0000x reference)
"""Tensor-parallel GQA attention block (AtlasAttentionWrapper) on 8 TRN2 cores.

Sharding: TP over heads. Core m owns query heads [4m..4m+3] (Wq rows
m*512:(m+1)*512), KV head m (Wk/Wv rows m*128:(m+1)*128, past_k/past_v head m)
and Wo columns m*512:(m+1)*512. Each core computes a full [1024, 4096] o_proj
partial; a ReduceScatter sums them and leaves rows [128m:128(m+1)] on core m;
the host concatenates the 8 row-shards.

Device layouts (host pre-transposes / pre-casts to bf16):
  xT   [4096, 1024]  = hidden^T         (replicated)
  wqT  [4096, 512]   = Wq_shard^T
  wkT  [4096, 128]   = Wk_shard^T
  wvT  [4096, 128]   = Wv_shard^T
  woT  [512, 4096]   = Wo[:, cols]^T
  pkT  [128, 1024]   = past_k[0,m]^T    (d, s)
  pv   [1024, 128]   = past_v[0,m]      (s, d)
  cosk/sink [128, 2048] = rope tables^T for all kv positions

All matmuls run bf16 with f32 PSUM accumulation. Scores are built
transposed (S^T[kv, q] = K Q^T) so softmax numerator exp() lands directly in
the [kv, q] layout the PV matmul consumes; the softmax denominator comes from
a ones-row matmul (also gives it pre-broadcast across partitions), and the
1/sum normalization is fused into the PSUM->SBUF copy of attn^T.
No max-subtraction: score scale is ~N(0, 1.7), exp() is safe in f32/bf16.
"""

import sys

if "/opt/trn_rl_repo" not in sys.path:
    sys.path.insert(0, "/opt/trn_rl_repo")

from contextlib import ExitStack

import ml_dtypes
import numpy as np

import concourse.bass as bass
import concourse.tile as tile
from concourse import bacc, mybir
from concourse.bass import ds, ts
from concourse.bass_utils import run_bass_kernel_spmd
from concourse.masks import make_identity

NCORES = 8
B, SQ, H = 1, 1024, 4096
NH, NKV, D = 32, 8, 128
SP = 1024
KV = SP + SQ  # 2048
HPC = NH // NCORES  # 4 query heads per core
DQ = HPC * D  # 512
SH = SQ // NCORES  # 128 output rows per core after ReduceScatter
ROPE_THETA = 10000.0
INV_SQRT_D = 1.0 / float(np.sqrt(D))

BF16 = mybir.dt.bfloat16
F32 = mybir.dt.float32
HCH = H // 128  # 32 contraction chunks
KVCH = KV // 128  # 16 kv chunks
NCHUNK = 4  # ReduceScatter chunks (overlap comm with o_proj)
EXP = mybir.ActivationFunctionType.Exp

LAST_RESULT = None
_NC_CACHE = {}


def _rope_write(nc, tmp_pool, dst, src, cos_sb, sin_sb, pos, width):
    """dst[d, s] = rope(src)[d, s] for s in [pos, pos+width) absolute positions.

    src: AP [128, width] (PSUM f32 or SBUF bf16), dst: SBUF bf16 AP.
    rope: out[d<64] = x[d]*cos[d] - x[d+64]*sin[d]
          out[d>=64] = x[d]*cos[d] + x[d-64]*sin[d]
    """
    cs = cos_sb[:, ds(pos, width)]
    sn = sin_sb[:, ds(pos, width)]
    t = tmp_pool.tile([128, width], F32, tag="rope_t")
    u = tmp_pool.tile([128, width], F32, tag="rope_u")
    nc.vector.tensor_mul(t[0:64, :], src[64:128, :], sn[0:64, :])
    nc.vector.tensor_mul(t[64:128, :], src[0:64, :], sn[64:128, :])
    nc.vector.tensor_mul(u[:, :], src[:, :], cs)
    nc.vector.tensor_sub(dst[0:64, :], u[0:64, :], t[0:64, :])
    nc.vector.tensor_add(dst[64:128, :], u[64:128, :], t[64:128, :])


def _build_nc():
    nc = bacc.Bacc(None, target_bir_lowering=False, debug=False)

    xT = nc.declare_dram_parameter("xT", [H, SQ], BF16, False)
    wqT = nc.declare_dram_parameter("wqT", [H, DQ], BF16, False)
    wkT = nc.declare_dram_parameter("wkT", [H, D], BF16, False)
    wvT = nc.declare_dram_parameter("wvT", [H, D], BF16, False)
    woT = nc.declare_dram_parameter("woT", [DQ, H], BF16, False)
    pkT = nc.declare_dram_parameter("pkT", [D, SP], BF16, False)
    pv = nc.declare_dram_parameter("pv", [SP, D], BF16, False)
    cosk = nc.declare_dram_parameter("cosk", [D, KV], BF16, False)
    sink = nc.declare_dram_parameter("sink", [D, KV], BF16, False)
    out_ext = nc.declare_dram_parameter("out", [SH, H], BF16, True)

    with tile.TileContext(nc) as tc, ExitStack() as ctx:
        # ---- persistent SBUF residents (live across all phases)
        const = ctx.enter_context(tc.tile_pool(name="const", bufs=1))
        kT_sb = const.tile([128, KV], BF16)  # roped K^T  [d, kv]
        v_sb = const.tile([128, KVCH, D], BF16)  # V chunks [kv%128, chunk, d]
        qT_sb = const.tile([128, HPC, SQ], BF16)  # roped Q^T per head [d, h, s]
        attnT_sb = const.tile([128, HPC, SQ], BF16)  # attn^T [d, h, s]
        cos_sb = const.tile([128, KV], BF16)
        sin_sb = const.tile([128, KV], BF16)
        ident = const.tile([128, 128], BF16)
        ones_sb = const.tile([128, 128], BF16)

        nc.sync.dma_start(out=cos_sb[:, :], in_=cosk[:, :])
        nc.sync.dma_start(out=sin_sb[:, :], in_=sink[:, :])
        make_identity(nc, ident[:, :])
        nc.vector.memset(ones_sb[:, :], 1.0)
        for c in range(SP // 128):
            nc.sync.dma_start(out=v_sb[:, c, :], in_=pv[ts(c, 128), :])

        # ---- PSUM pools (8 banks total). Tags share slots across phases:
        # "st" [128,1024] f32 (2 banks) x2 bufs = 4 banks — proj matmuls,
        # score tiles, o_proj matmuls; "sums"/"att" [128,512] x2 = 2+2 banks.
        st_ps = ctx.enter_context(tc.tile_pool(name="st_ps", bufs=2, space="PSUM"))
        sums_ps = ctx.enter_context(tc.tile_pool(name="sums_ps", bufs=2, space="PSUM"))
        at_ps = ctx.enter_context(tc.tile_pool(name="at_ps", bufs=2, space="PSUM"))

        rope_tmp = ctx.enter_context(tc.tile_pool(name="rope_tmp", bufs=2))
        dram = ctx.enter_context(tc.tile_pool(name="dram", bufs=1, space="DRAM"))
        # 4 RS chunks of 256 seq rows each; core m gets rows 32m:32m+32 of each
        # chunk's sum (host reassembles the interleaving)
        part_chunks = []
        rs_chunks = []
        for k in range(NCHUNK):
            part_chunks.append(
                dram.tile([SQ // NCHUNK, H], BF16, tag=f"part{k}", name=f"part{k}")
            )
            rs_chunks.append(
                dram.tile(
                    [SQ // NCHUNK // NCORES, H], BF16, tag=f"rs{k}", name=f"rs{k}"
                )
            )

        # ================= Phase 1: projections + rope ==================
        with tc.tile_pool(name="proj", bufs=1) as proj:
            xT_sb = proj.tile([128, HCH, SQ], BF16)
            wqT_sb = proj.tile([128, HCH, DQ], BF16)
            wkT_sb = proj.tile([128, HCH, D], BF16)
            wvT_sb = proj.tile([128, HCH, D], BF16)

            # DMA priority: K/V weights + xT first (K/V proj gates attention),
            # Wq after.
            for c in range(HCH):
                nc.sync.dma_start(out=wkT_sb[:, c, :], in_=wkT[ts(c, 128), :])
                nc.sync.dma_start(out=wvT_sb[:, c, :], in_=wvT[ts(c, 128), :])
                nc.sync.dma_start(out=xT_sb[:, c, :], in_=xT[ts(c, 128), :])
            # past_k arrives pre-roped (host-side) and transposed
            nc.sync.dma_start(out=kT_sb[:, 0:SP], in_=pkT[:, :])
            for c in range(HCH):
                nc.sync.dma_start(out=wqT_sb[:, c, :], in_=wqT[ts(c, 128), :])

            # K_new^T -> rope -> kT_sb[:, SP:]
            for g in range(SQ // 512):
                ps = st_ps.tile([128, 1024], F32, tag="st")
                for c in range(HCH):
                    nc.tensor.matmul(
                        ps[:, 0:512],
                        lhsT=wkT_sb[:, c, :],
                        rhs=xT_sb[:, c, ts(g, 512)],
                        start=(c == 0),
                        stop=(c == HCH - 1),
                    )
                _rope_write(
                    nc, rope_tmp, kT_sb[:, ds(SP + g * 512, 512)], ps[:, 0:512],
                    cos_sb, sin_sb, SP + g * 512, 512,
                )
            # V_new^T then transpose into v_sb chunks [SP/128 ..)
            with tc.tile_pool(name="vtmp", bufs=2) as vtmp:
                for g in range(SQ // 512):
                    ps = st_ps.tile([128, 1024], F32, tag="st")
                    for c in range(HCH):
                        nc.tensor.matmul(
                            ps[:, 0:512],
                            lhsT=wvT_sb[:, c, :],
                            rhs=xT_sb[:, c, ts(g, 512)],
                            start=(c == 0),
                            stop=(c == HCH - 1),
                        )
                    vt = vtmp.tile([128, 512], BF16)
                    nc.any.tensor_copy(vt[:, :], ps[:, 0:512])
                    for k in range(4):
                        ps2 = sums_ps.tile([128, 128], BF16, tag="sums")
                        nc.tensor.transpose(ps2[:, :], vt[:, ts(k, 128)], ident[:, :])
                        nc.any.tensor_copy(
                            v_sb[:, SP // 128 + g * 4 + k, :], ps2[:, :]
                        )

            # Q^T[d, h, s] = Wq_h^T.T @ X^T, then rope (query pos = SP + s)
            for j in range(HPC):
                for g in range(SQ // 512):
                    ps = st_ps.tile([128, 1024], F32, tag="st")
                    for c in range(HCH):
                        nc.tensor.matmul(
                            ps[:, 0:512],
                            lhsT=wqT_sb[:, c, ts(j, 128)],
                            rhs=xT_sb[:, c, ts(g, 512)],
                            start=(c == 0),
                            stop=(c == HCH - 1),
                        )
                    _rope_write(
                        nc, rope_tmp, qT_sb[:, j, ts(g, 512)], ps[:, 0:512],
                        cos_sb, sin_sb, SP + g * 512, 512,
                    )

        # ================= Phase 2: attention ==================
        # Per (head, kv-chunk): one [kv=128, q=1024] score tile (both q-halves
        # share the kT stationary and one wide exp), then ones-sums and PV
        # accumulate per q-half.
        pt_pool = ctx.enter_context(tc.tile_pool(name="pt", bufs=4))
        rc_pool = ctx.enter_context(tc.tile_pool(name="rc", bufs=2))
        for h in range(HPC):
            sums = []
            att = []
            for g in range(SQ // 512):
                sums.append(sums_ps.tile([128, 512], F32, tag="sums", name=f"sums{h}{g}"))
                att.append(at_ps.tile([128, 512], F32, tag="att", name=f"att{h}{g}"))
            for c in range(KVCH):
                st = st_ps.tile([128, 1024], F32, tag="st")
                for g in range(SQ // 512):
                    nc.tensor.matmul(
                        st[:, ts(g, 512)],
                        lhsT=kT_sb[:, ts(c, 128)],
                        rhs=qT_sb[:, h, ts(g, 512)],
                        start=True,
                        stop=True,
                    )
                pt = pt_pool.tile([128, 1024], BF16)
                nc.scalar.activation(pt[:, :], st[:, :], EXP, scale=INV_SQRT_D)
                for g in range(SQ // 512):
                    nc.tensor.matmul(
                        sums[g][:, :],
                        lhsT=ones_sb[:, :],
                        rhs=pt[:, ts(g, 512)],
                        start=(c == 0),
                        stop=(c == KVCH - 1),
                    )
                for g in range(SQ // 512):
                    nc.tensor.matmul(
                        att[g][:, :],
                        lhsT=v_sb[:, c, :],
                        rhs=pt[:, ts(g, 512)],
                        start=(c == 0),
                        stop=(c == KVCH - 1),
                    )
            for g in range(SQ // 512):
                recip = rc_pool.tile([128, 512], F32)
                nc.vector.reciprocal(recip[:, :], sums[g][:, :])
                nc.vector.tensor_mul(
                    attnT_sb[:, h, ts(g, 512)], att[g][:, :], recip[:, :]
                )

        # ================= Phase 3: o_proj + ReduceScatter ==================
        with (
            tc.tile_pool(name="wo", bufs=1) as wo_pool,
            tc.tile_pool(name="ob", bufs=4) as ob_pool,
        ):
            wo_sb = wo_pool.tile([128, HPC, H], BF16)
            for j in range(HPC):
                nc.sync.dma_start(out=wo_sb[:, j, :], in_=woT[ts(j, 128), :])

            spc = SQ // NCHUNK // 128  # s-tiles per RS chunk
            rsh = SQ // NCHUNK // NCORES  # rows per core per RS chunk
            for k in range(NCHUNK):
                for ii in range(spc):
                    i = k * spc + ii
                    for n in range(H // 512):
                        ps = st_ps.tile([128, 512], F32, tag="st")
                        for j in range(HPC):
                            nc.tensor.matmul(
                                ps[:, :],
                                lhsT=attnT_sb[:, j, ts(i, 128)],
                                rhs=wo_sb[:, j, ts(n, 512)],
                                start=(j == 0),
                                stop=(j == HPC - 1),
                            )
                        ob = ob_pool.tile([128, 512], BF16)
                        nc.any.tensor_copy(ob[:, :], ps[:, :])
                        nc.sync.dma_start(
                            out=part_chunks[k][ts(ii, 128), ts(n, 512)], in_=ob[:, :]
                        )
                nc.gpsimd.collective_compute(
                    "ReduceScatter",
                    mybir.AluOpType.add,
                    ins=[part_chunks[k][:, :].opt()],
                    outs=[rs_chunks[k][:, :].opt()],
                    replica_groups=[list(range(NCORES))],
                )
                nc.sync.dma_start(
                    out=out_ext[ts(k, rsh), :], in_=rs_chunks[k][:, :]
                )

    nc.finalize()
    return nc


def _get_nc():
    if "nc" not in _NC_CACHE:
        _NC_CACHE["nc"] = _build_nc()
    return _NC_CACHE["nc"]


def _rope_tables():
    inv_freq = 1.0 / (ROPE_THETA ** (np.arange(0, D, 2, dtype=np.float32) / D))
    pos = np.arange(KV, dtype=np.float32)
    freqs = pos[:, None] * inv_freq[None, :]  # [KV, D/2]
    emb = np.concatenate([freqs, freqs], axis=-1)  # [KV, D]
    return np.cos(emb), np.sin(emb)  # [KV, D]


def _host_rope(x, cos, sin):
    # x: [S, D]; cos/sin: [S, D]
    x1, x2 = x[:, : D // 2], x[:, D // 2 :]
    rot = np.concatenate([-x2, x1], axis=-1)
    return x * cos + rot * sin


def kernel(hidden_states, past_k, past_v, Wq, Wk, Wv, Wo, trace=False):
    global LAST_RESULT
    bf = ml_dtypes.bfloat16
    x = np.asarray(hidden_states, dtype=np.float32)[0]  # [SQ, H]
    xT = np.ascontiguousarray(x.T).astype(bf)
    cos, sin = _rope_tables()  # [KV, D] f32
    cosT = np.ascontiguousarray(cos.T).astype(bf)
    sinT = np.ascontiguousarray(sin.T).astype(bf)

    in_maps = []
    for m in range(NCORES):
        qr = slice(m * DQ, (m + 1) * DQ)
        kr = slice(m * D, (m + 1) * D)
        in_maps.append(
            {
                "xT": xT,
                "wqT": np.ascontiguousarray(np.asarray(Wq)[qr].T).astype(bf),
                "wkT": np.ascontiguousarray(np.asarray(Wk)[kr].T).astype(bf),
                "wvT": np.ascontiguousarray(np.asarray(Wv)[kr].T).astype(bf),
                "woT": np.ascontiguousarray(np.asarray(Wo)[:, qr].T).astype(bf),
                "pkT": np.ascontiguousarray(
                    _host_rope(
                        np.asarray(past_k, dtype=np.float32)[0, m], cos[:SP], sin[:SP]
                    ).T
                ).astype(bf),
                "pv": np.ascontiguousarray(np.asarray(past_v)[0, m]).astype(bf),
                "cosk": cosT,
                "sink": sinT,
            }
        )

    nc = _get_nc()
    res = run_bass_kernel_spmd(
        nc, in_maps, core_ids=list(range(NCORES)), trace=trace
    )
    LAST_RESULT = res
    # Each core's "out" holds NCHUNK blocks of rsh rows; block k of core m is
    # global rows [csz*k + rsh*m, csz*k + rsh*(m+1)).
    csz = SQ // NCHUNK
    rsh = csz // NCORES
    out = np.empty((SQ, H), dtype=np.float32)
    for m in range(NCORES):
        shard = np.asarray(res.results[m]["out"], dtype=np.float32)
        for k in range(NCHUNK):
            out[csz * k + rsh * m : csz * k + rsh * (m + 1)] = shard[
                rsh * k : rsh * (k + 1)
            ]
    return out.reshape(B, SQ, H)


# revision 21
# speedup vs baseline: 1.1888x; 1.1888x over previous
"""Tensor-parallel GQA attention block (AtlasAttentionWrapper) on 8 TRN2 cores.

Sharding: TP over heads. Core m owns query heads [4m..4m+3] (Wq rows
m*512:(m+1)*512), KV head m (Wk/Wv rows m*128:(m+1)*128, past_k/past_v head m)
and Wo columns m*512:(m+1)*512. Each core computes a full [1024, 4096] o_proj
partial; a ReduceScatter sums them and leaves rows [128m:128(m+1)] on core m;
the host concatenates the 8 row-shards.

Device layouts (host pre-transposes / pre-casts to bf16):
  xT   [4096, 1024]  = hidden^T         (replicated)
  wqT  [4096, 512]   = Wq_shard^T
  wkT  [4096, 128]   = Wk_shard^T
  wvT  [4096, 128]   = Wv_shard^T
  woT  [512, 4096]   = Wo[:, cols]^T
  pkT  [128, 1024]   = past_k[0,m]^T    (d, s)
  pv   [1024, 128]   = past_v[0,m]      (s, d)
  cosk/sink [128, 2048] = rope tables^T for all kv positions

All matmuls run bf16 with f32 PSUM accumulation. Scores are built
transposed (S^T[kv, q] = K Q^T) so softmax numerator exp() lands directly in
the [kv, q] layout the PV matmul consumes; the softmax denominator comes from
a ones-row matmul (also gives it pre-broadcast across partitions), and the
1/sum normalization is fused into the PSUM->SBUF copy of attn^T.
No max-subtraction: score scale is ~N(0, 1.7), exp() is safe in f32/bf16.
"""

import sys

if "/opt/trn_rl_repo" not in sys.path:
    sys.path.insert(0, "/opt/trn_rl_repo")

from contextlib import ExitStack

import ml_dtypes
import numpy as np

import concourse.bass as bass
import concourse.tile as tile
from concourse import bacc, mybir
from concourse.bass import ds, ts
from concourse.bass_utils import run_bass_kernel_spmd
from concourse.masks import make_identity

NCORES = 8
B, SQ, H = 1, 1024, 4096
NH, NKV, D = 32, 8, 128
SP = 1024
KV = SP + SQ  # 2048
HPC = NH // NCORES  # 4 query heads per core
DQ = HPC * D  # 512
SH = SQ // NCORES  # 128 output rows per core after ReduceScatter
ROPE_THETA = 10000.0
INV_SQRT_D = 1.0 / float(np.sqrt(D))

BF16 = mybir.dt.bfloat16
F32 = mybir.dt.float32
HCH = H // 128  # 32 contraction chunks
KVCH = KV // 128  # 16 kv chunks
NCHUNK = 4  # ReduceScatter chunks (overlap comm with o_proj)
EXP = mybir.ActivationFunctionType.Exp

LAST_RESULT = None
_NC_CACHE = {}


def _rope_write(nc, tmp_pool, dst, src, cos_sb, sin_sb, pos, width):
    """dst[d, s] = rope(src)[d, s] for s in [pos, pos+width) absolute positions.

    src: AP [128, width] (PSUM f32 or SBUF bf16), dst: SBUF bf16 AP.
    rope: out[d<64] = x[d]*cos[d] - x[d+64]*sin[d]
          out[d>=64] = x[d]*cos[d] + x[d-64]*sin[d]
    """
    cs = cos_sb[:, ds(pos, width)]
    sn = sin_sb[:, ds(pos, width)]
    t = tmp_pool.tile([128, width], F32, tag="rope_t")
    u = tmp_pool.tile([128, width], F32, tag="rope_u")
    nc.vector.tensor_mul(t[0:64, :], src[64:128, :], sn[0:64, :])
    nc.vector.tensor_mul(t[64:128, :], src[0:64, :], sn[64:128, :])
    nc.vector.tensor_mul(u[:, :], src[:, :], cs)
    nc.vector.tensor_sub(dst[0:64, :], u[0:64, :], t[0:64, :])
    nc.vector.tensor_add(dst[64:128, :], u[64:128, :], t[64:128, :])


def _build_nc():
    nc = bacc.Bacc(None, target_bir_lowering=False, debug=False)

    xT = nc.declare_dram_parameter("xT", [H, SQ], BF16, False)
    wqT = nc.declare_dram_parameter("wqT", [H, DQ], BF16, False)
    wkT = nc.declare_dram_parameter("wkT", [H, D], BF16, False)
    wvT = nc.declare_dram_parameter("wvT", [H, D], BF16, False)
    woT = nc.declare_dram_parameter("woT", [DQ, H], BF16, False)
    pkT = nc.declare_dram_parameter("pkT", [D, SP], BF16, False)
    pv = nc.declare_dram_parameter("pv", [SP, D], BF16, False)
    cosk = nc.declare_dram_parameter("cosk", [D, KV], BF16, False)
    sink = nc.declare_dram_parameter("sink", [D, KV], BF16, False)
    out_ext = nc.declare_dram_parameter("out", [SH, H], BF16, True)

    with tile.TileContext(nc) as tc, ExitStack() as ctx:
        # ---- persistent SBUF residents (live across all phases)
        const = ctx.enter_context(tc.tile_pool(name="const", bufs=1))
        kT_sb = const.tile([128, KV], BF16)  # roped K^T  [d, kv]
        v_sb = const.tile([128, KVCH, D], BF16)  # V chunks [kv%128, chunk, d]
        qT_sb = const.tile([128, HPC, SQ], BF16)  # roped Q^T per head [d, h, s]
        attnT_sb = const.tile([128, HPC, SQ], BF16)  # attn^T [d, h, s]
        cos_sb = const.tile([128, KV], BF16)
        sin_sb = const.tile([128, KV], BF16)
        ident = const.tile([128, 128], BF16)
        ones_sb = const.tile([128, 128], BF16)

        nc.sync.dma_start(out=cos_sb[:, :], in_=cosk[:, :])
        nc.sync.dma_start(out=sin_sb[:, :], in_=sink[:, :])
        make_identity(nc, ident[:, :])
        nc.vector.memset(ones_sb[:, :], 1.0)
        for c in range(SP // 128):
            nc.sync.dma_start(out=v_sb[:, c, :], in_=pv[ts(c, 128), :])

        # ---- PSUM pools (8 banks total). Tags share slots across phases:
        # "st" [128,512] x2 = 2 banks (proj matmuls, score tiles, o_proj),
        # "sums" x3 = 3, "att" x3 = 3. The 3-deep sums/att pools keep the
        # reciprocal+normalize (DVE) off the PE critical path at head
        # boundaries — PE advances into the next head with the spare slot.
        st_ps = ctx.enter_context(tc.tile_pool(name="st_ps", bufs=2, space="PSUM"))
        sums_ps = ctx.enter_context(tc.tile_pool(name="sums_ps", bufs=3, space="PSUM"))
        at_ps = ctx.enter_context(tc.tile_pool(name="at_ps", bufs=3, space="PSUM"))

        rope_tmp = ctx.enter_context(tc.tile_pool(name="rope_tmp", bufs=2))
        dram = ctx.enter_context(tc.tile_pool(name="dram", bufs=1, space="DRAM"))
        # 4 RS chunks of 256 seq rows each; core m gets rows 32m:32m+32 of each
        # chunk's sum (host reassembles the interleaving)
        part_chunks = []
        rs_chunks = []
        for k in range(NCHUNK):
            part_chunks.append(
                dram.tile([SQ // NCHUNK, H], BF16, tag=f"part{k}", name=f"part{k}")
            )
            rs_chunks.append(
                dram.tile(
                    [SQ // NCHUNK // NCORES, H], BF16, tag=f"rs{k}", name=f"rs{k}"
                )
            )

        # ================= Phase 1: projections + rope ==================
        with tc.tile_pool(name="proj", bufs=1) as proj:
            xT_sb = proj.tile([128, HCH, SQ], BF16)
            wqT_sb = proj.tile([128, HCH, DQ], BF16)
            wkT_sb = proj.tile([128, HCH, D], BF16)
            wvT_sb = proj.tile([128, HCH, D], BF16)

            # DMA priority: K/V weights + xT first (K/V proj gates attention),
            # Wq after.
            for c in range(HCH):
                nc.sync.dma_start(out=wkT_sb[:, c, :], in_=wkT[ts(c, 128), :])
                nc.sync.dma_start(out=wvT_sb[:, c, :], in_=wvT[ts(c, 128), :])
                nc.sync.dma_start(out=xT_sb[:, c, :], in_=xT[ts(c, 128), :])
            # past_k arrives pre-roped (host-side) and transposed
            nc.sync.dma_start(out=kT_sb[:, 0:SP], in_=pkT[:, :])
            for c in range(HCH):
                nc.sync.dma_start(out=wqT_sb[:, c, :], in_=wqT[ts(c, 128), :])

            # K_new^T -> rope -> kT_sb[:, SP:]
            for g in range(SQ // 512):
                ps = st_ps.tile([128, 512], F32, tag="st")
                for c in range(HCH):
                    nc.tensor.matmul(
                        ps[:, :],
                        lhsT=wkT_sb[:, c, :],
                        rhs=xT_sb[:, c, ts(g, 512)],
                        start=(c == 0),
                        stop=(c == HCH - 1),
                    )
                _rope_write(
                    nc, rope_tmp, kT_sb[:, ds(SP + g * 512, 512)], ps[:, :],
                    cos_sb, sin_sb, SP + g * 512, 512,
                )
            # V_new^T then transpose into v_sb chunks [SP/128 ..)
            with tc.tile_pool(name="vtmp", bufs=2) as vtmp:
                for g in range(SQ // 512):
                    ps = st_ps.tile([128, 512], F32, tag="st")
                    for c in range(HCH):
                        nc.tensor.matmul(
                            ps[:, :],
                            lhsT=wvT_sb[:, c, :],
                            rhs=xT_sb[:, c, ts(g, 512)],
                            start=(c == 0),
                            stop=(c == HCH - 1),
                        )
                    vt = vtmp.tile([128, 512], BF16)
                    nc.any.tensor_copy(vt[:, :], ps[:, :])
                    for k in range(4):
                        ps2 = sums_ps.tile([128, 128], BF16, tag="sums")
                        nc.tensor.transpose(ps2[:, :], vt[:, ts(k, 128)], ident[:, :])
                        nc.any.tensor_copy(
                            v_sb[:, SP // 128 + g * 4 + k, :], ps2[:, :]
                        )

            # Q^T[d, h, s] = Wq_h^T.T @ X^T, then rope (query pos = SP + s)
            for j in range(HPC):
                for g in range(SQ // 512):
                    ps = st_ps.tile([128, 512], F32, tag="st")
                    for c in range(HCH):
                        nc.tensor.matmul(
                            ps[:, :],
                            lhsT=wqT_sb[:, c, ts(j, 128)],
                            rhs=xT_sb[:, c, ts(g, 512)],
                            start=(c == 0),
                            stop=(c == HCH - 1),
                        )
                    _rope_write(
                        nc, rope_tmp, qT_sb[:, j, ts(g, 512)], ps[:, :],
                        cos_sb, sin_sb, SP + g * 512, 512,
                    )

        # ============ Phase 2+3 interleaved: attention, o_proj, RS ==========
        # Attention runs g-outer (all heads for q-half g), so the o_proj +
        # ReduceScatter of q-half 0 overlaps the attention of q-half 1.
        pt_pool = ctx.enter_context(tc.tile_pool(name="pt", bufs=4))
        rc_pool = ctx.enter_context(tc.tile_pool(name="rc", bufs=2))
        wo_pool = ctx.enter_context(tc.tile_pool(name="wo", bufs=1))
        ob_pool = ctx.enter_context(tc.tile_pool(name="ob", bufs=4))
        wo_sb = wo_pool.tile([128, HPC, H], BF16)
        for j in range(HPC):
            nc.sync.dma_start(out=wo_sb[:, j, :], in_=woT[ts(j, 128), :])

        spc = SQ // NCHUNK // 128  # s-tiles per RS chunk
        rsh = SQ // NCHUNK // NCORES  # rows per core per RS chunk

        def attention_half(g):
            for h in range(HPC):
                sums = sums_ps.tile([128, 512], F32, tag="sums", name=f"sums{h}{g}")
                att = at_ps.tile([128, 512], F32, tag="att", name=f"att{h}{g}")
                for c in range(KVCH):
                    st = st_ps.tile([128, 512], F32, tag="st", name="st")
                    nc.tensor.matmul(
                        st[:, :],
                        lhsT=kT_sb[:, ts(c, 128)],
                        rhs=qT_sb[:, h, ts(g, 512)],
                        start=True,
                        stop=True,
                    )
                    pt = pt_pool.tile([128, 512], BF16, name="pt")
                    nc.scalar.activation(pt[:, :], st[:, :], EXP, scale=INV_SQRT_D)
                    nc.tensor.matmul(
                        sums[:, :],
                        lhsT=ones_sb[:, :],
                        rhs=pt[:, :],
                        start=(c == 0),
                        stop=(c == KVCH - 1),
                    )
                    nc.tensor.matmul(
                        att[:, :],
                        lhsT=v_sb[:, c, :],
                        rhs=pt[:, :],
                        start=(c == 0),
                        stop=(c == KVCH - 1),
                    )
                recip = rc_pool.tile([128, 512], F32, name="recip")
                nc.vector.reciprocal(recip[:, :], sums[:, :])
                nc.vector.tensor_mul(
                    attnT_sb[:, h, ts(g, 512)], att[:, :], recip[:, :]
                )

        def oproj_chunk(k):
            for ii in range(spc):
                i = k * spc + ii
                for n in range(H // 512):
                    ps = st_ps.tile([128, 512], F32, tag="st", name="ops")
                    for j in range(HPC):
                        nc.tensor.matmul(
                            ps[:, :],
                            lhsT=attnT_sb[:, j, ts(i, 128)],
                            rhs=wo_sb[:, j, ts(n, 512)],
                            start=(j == 0),
                            stop=(j == HPC - 1),
                        )
                    ob = ob_pool.tile([128, 512], BF16, name="ob")
                    nc.any.tensor_copy(ob[:, :], ps[:, :])
                    nc.sync.dma_start(
                        out=part_chunks[k][ts(ii, 128), ts(n, 512)], in_=ob[:, :]
                    )
            nc.gpsimd.collective_compute(
                "ReduceScatter",
                mybir.AluOpType.add,
                ins=[part_chunks[k][:, :].opt()],
                outs=[rs_chunks[k][:, :].opt()],
                replica_groups=[list(range(NCORES))],
            )
            nc.sync.dma_start(out=out_ext[ts(k, rsh), :], in_=rs_chunks[k][:, :])

        attention_half(0)
        oproj_chunk(0)
        oproj_chunk(1)
        attention_half(1)
        oproj_chunk(2)
        oproj_chunk(3)

    nc.finalize()
    return nc


def _get_nc():
    if "nc" not in _NC_CACHE:
        _NC_CACHE["nc"] = _build_nc()
    return _NC_CACHE["nc"]


def _rope_tables():
    inv_freq = 1.0 / (ROPE_THETA ** (np.arange(0, D, 2, dtype=np.float32) / D))
    pos = np.arange(KV, dtype=np.float32)
    freqs = pos[:, None] * inv_freq[None, :]  # [KV, D/2]
    emb = np.concatenate([freqs, freqs], axis=-1)  # [KV, D]
    return np.cos(emb), np.sin(emb)  # [KV, D]


def _host_rope(x, cos, sin):
    # x: [S, D]; cos/sin: [S, D]
    x1, x2 = x[:, : D // 2], x[:, D // 2 :]
    rot = np.concatenate([-x2, x1], axis=-1)
    return x * cos + rot * sin


def kernel(hidden_states, past_k, past_v, Wq, Wk, Wv, Wo, trace=False):
    global LAST_RESULT
    bf = ml_dtypes.bfloat16
    x = np.asarray(hidden_states, dtype=np.float32)[0]  # [SQ, H]
    xT = np.ascontiguousarray(x.T).astype(bf)
    cos, sin = _rope_tables()  # [KV, D] f32
    cosT = np.ascontiguousarray(cos.T).astype(bf)
    sinT = np.ascontiguousarray(sin.T).astype(bf)

    in_maps = []
    for m in range(NCORES):
        qr = slice(m * DQ, (m + 1) * DQ)
        kr = slice(m * D, (m + 1) * D)
        in_maps.append(
            {
                "xT": xT,
                "wqT": np.ascontiguousarray(np.asarray(Wq)[qr].T).astype(bf),
                "wkT": np.ascontiguousarray(np.asarray(Wk)[kr].T).astype(bf),
                "wvT": np.ascontiguousarray(np.asarray(Wv)[kr].T).astype(bf),
                "woT": np.ascontiguousarray(np.asarray(Wo)[:, qr].T).astype(bf),
                "pkT": np.ascontiguousarray(
                    _host_rope(
                        np.asarray(past_k, dtype=np.float32)[0, m], cos[:SP], sin[:SP]
                    ).T
                ).astype(bf),
                "pv": np.ascontiguousarray(np.asarray(past_v)[0, m]).astype(bf),
                "cosk": cosT,
                "sink": sinT,
            }
        )

    nc = _get_nc()
    res = run_bass_kernel_spmd(
        nc, in_maps, core_ids=list(range(NCORES)), trace=trace
    )
    LAST_RESULT = res
    # Each core's "out" holds NCHUNK blocks of rsh rows; block k of core m is
    # global rows [csz*k + rsh*m, csz*k + rsh*(m+1)).
    csz = SQ // NCHUNK
    rsh = csz // NCORES
    out = np.empty((SQ, H), dtype=np.float32)
    for m in range(NCORES):
        shard = np.asarray(res.results[m]["out"], dtype=np.float32)
        for k in range(NCHUNK):
            out[csz * k + rsh * m : csz * k + rsh * (m + 1)] = shard[
                rsh * k : rsh * (k + 1)
            ]
    return out.reshape(B, SQ, H)
